# revision 21
# baseline (speedup 1.0000x reference)
# Bidirectional LSTM (T=128, B=128, NIN=NH=NOUT=512) on 8 trn2 NeuronCores.
#
# Sharding: 8 cores = 2 directions x 4 batch-quarters (B_loc=32). Fully
# symmetric SPMD program (no collectives): reverse-direction cores receive
# time-flipped inputs; the host flips their outputs back and sums the two
# directions' output-projection partials (+ b_emb).
#
# Per-core program:
#   phase 1: xp[t] = W_ih @ x_t + (b_ih + b_hh)  for all t  (bf16 matmuls,
#            f32 accum, spilled to an internal DRAM buffer)
#   phase 2: 128-step recurrence. Gates for step t are accumulated in PSUM:
#            an identity-weight matmul seeds xp_t, then 64 small matmuls add
#            W_hh @ h_{t-1}. Gates are split into two half-banks so ACT/DVE
#            elementwise for one half overlaps PE matmuls for the other.
#   phase 3: out_partial = W_emb_dir @ h  from the SBUF-resident h history.
import os
import sys

for _p in ("/opt/trn_rl_repo", "/root/.axon_site/_ro/trn_rl_repo"):
    if os.path.isdir(_p) and _p not in sys.path:
        sys.path.insert(0, _p)

import ml_dtypes
import numpy as np

import concourse.bass as bass  # noqa: F401  (registers bass types)
import concourse.mybir as mybir
import concourse.tile as tile
from concourse import bacc
from concourse.bass_utils import run_bass_kernel_spmd

BF16_NP = ml_dtypes.bfloat16
F8_NP = ml_dtypes.float8_e4m3
F32 = mybir.dt.float32
BF16 = mybir.dt.bfloat16
FP8 = mybir.dt.float8e4
AF = mybir.ActivationFunctionType

T = 128
B = 128
NIN = 512
NH = 512
G = 4 * NH  # 2048 gate rows
NOUT = 512
NCORES = 8
BL = B // 4  # 32 batch rows per core (4 quarters x 2 directions)
TOK = T * BL  # 4096 tokens per core
NB = TOK // 512  # 8 token blocks of 512
KCH = NIN // 128  # 4 contraction chunks
MT = G // 128  # 16 gate M-tiles

# Gate-region permutation: PyTorch rows are (i, f, g, o); we lay out gate
# M-tiles in region order (i, f, o, g) so sigmoid covers one contiguous
# range and tanh another.
_PERM = np.r_[0:512, 512:1024, 1536:2048, 1024:1536]


def _build_program(floor=False, variant="full", loop_repeat=None, v2=False, v3=False,
                   v4=False):
    nc = bacc.Bacc("TRN2", target_bir_lowering=False, debug=False)
    xT_d = nc.dram_tensor("xT", [NIN, TOK], BF16, kind="ExternalInput").ap()
    wi_d = nc.dram_tensor("wiT", [NIN, G], BF16, kind="ExternalInput").ap()
    wh_d = nc.dram_tensor("whT", [NH, G], BF16, kind="ExternalInput").ap()
    we_d = nc.dram_tensor("weT", [NH, NOUT], BF16, kind="ExternalInput").ap()
    b_d = nc.dram_tensor("biasM", [128, MT], F32, kind="ExternalInput").ap()
    i_d = nc.dram_tensor("ident", [128, 128], BF16, kind="ExternalInput").ap()
    out_d = nc.dram_tensor("outT", [NOUT, TOK], F32, kind="ExternalOutput").ap()

    if floor:
        # Same external I/O, near-empty body: used by test.py to measure the
        # per-call dispatch floor so kernel HW time = full - floor.
        with tile.TileContext(nc) as tc:
            with tc.tile_pool(name="f", bufs=1) as fp:
                t_ = fp.tile([128, 512], BF16, tag="t")
                nc.sync.dma_start(out=t_, in_=xT_d[0:128, 0:512])
                t2_ = fp.tile([128, 512], F32, tag="t2")
                nc.vector.tensor_copy(t2_, t_)
                nc.sync.dma_start(out=out_d[0:128, 0:512], in_=t2_)
        nc.compile()
        return nc

    with tile.TileContext(nc) as tc:
        with tc.tile_pool(name="consts", bufs=1) as consts, \
             tc.tile_pool(name="xtk", bufs=2) as xtkp, \
             tc.tile_pool(name="xpst", bufs=4) as xpstp, \
             tc.tile_pool(name="gsb", bufs=3 if (v2 or v3) else 2) as gsbp, \
             tc.tile_pool(name="xpch", bufs=6 if (v2 or v3) else 3) as xpchp, \
             tc.tile_pool(name="small", bufs=3 if (v2 or v3) else 2) as smallp, \
             tc.tile_pool(name="osb", bufs=3) as osbp, \
             tc.tile_pool(name="psmm", bufs=2 if v2 else 4, space="PSUM") as psmm, \
             tc.tile_pool(name="psch", bufs=3 if v2 else 2, space="PSUM") as psch, \
             tc.tile_pool(name="xpdram", bufs=1, space="DRAM") as xpdr:

            # ---- constant loads ----
            wi_sb = consts.tile([128, KCH, G], BF16, tag="wi")
            nc.sync.dma_start(out=wi_sb, in_=wi_d.rearrange("(k p) m -> p k m", p=128))
            wh_sb = consts.tile([128, KCH, G], BF16, tag="wh")
            nc.sync.dma_start(out=wh_sb, in_=wh_d.rearrange("(k p) m -> p k m", p=128))
            we_sb = consts.tile([128, KCH, NOUT], BF16, tag="we")
            nc.sync.dma_start(out=we_sb, in_=we_d.rearrange("(j p) o -> p j o", p=128))
            b_sb = consts.tile([128, MT], F32, tag="bias")
            nc.sync.dma_start(out=b_sb, in_=b_d)
            id_sb = consts.tile([128, 128], BF16, tag="ident")
            nc.sync.dma_start(out=id_sb, in_=i_d)
            hh = consts.tile([128, T * 128], BF16, tag="hh")  # h history
            if variant in ("p3", "p13"):
                # phase-2 (the hh producer) is ablated away; give hh a
                # one-time producer outside the timing loop
                nc.vector.memset(hh, 0.25)

            # xp spill, laid out so the chain's per-step gather is a 3-dim
            # slice: [half][partition][g*2+j'][token]  (m-tile m = g*4+2*half+j')
            xp_dram = xpdr.tile([2, 128, 8, TOK], BF16, tag="xp")

            # Optional hardware loop around the whole body (timing
            # amplification for benchmarking: one NEFF runs the body R times).
            import contextlib
            rep_cm = tc.For_i(0, loop_repeat, 1) if loop_repeat else contextlib.nullcontext()
            with rep_cm:
                _emit_body(nc, tc, variant, locals(), v2=v2, v4=v4)
    nc.compile()
    return nc


def _emit_body(nc, tc, variant, env, v2=False, v4=False):
    xT_d = env["xT_d"]
    wi_sb = env["wi_sb"]; wh_sb = env["wh_sb"]; we_sb = env["we_sb"]
    b_sb = env["b_sb"]; id_sb = env["id_sb"]; hh = env["hh"]
    xp_dram = env["xp_dram"]; out_d = env["out_d"]
    xtkp = env["xtkp"]; xpstp = env["xpstp"]; gsbp = env["gsbp"]
    xpchp = env["xpchp"]; smallp = env["smallp"]; osbp = env["osbp"]
    psmm = env["psmm"]; psch = env["psch"]
    # Phase-subset ablation variants: "p1", "p2", "p3", "p13", "p12"
    do_p1 = variant not in ("p2", "p3")
    do_p2 = variant not in ("p1", "p3", "p13")
    do_p3 = variant not in ("p1", "p2", "p12")
    if True:
            # ---- phase 1: input projections ----
            xT_r = xT_d.rearrange("(k p) t -> p k t", p=128)
            for nb in range(NB if do_p1 else 0):
                xt = xtkp.tile([128, KCH, 512], BF16, tag="xt")
                nc.sync.dma_start(out=xt, in_=xT_r[:, :, 512 * nb:512 * (nb + 1)])
                for m in range(MT):
                    psx = psmm.tile([128, 512], F32, tag="psmm")
                    for k in range(KCH):
                        nc.tensor.matmul(psx, wi_sb[:, k, 128 * m:128 * (m + 1)],
                                         xt[:, k, :], start=(k == 0), stop=(k == KCH - 1))
                    xpt = xpstp.tile([128, 512], BF16, tag="xpst")
                    if m % 2 == 0:
                        nc.scalar.activation(xpt, psx, AF.Identity, bias=b_sb[:, m:m + 1])
                    else:
                        nc.vector.tensor_scalar_add(xpt, psx, b_sb[:, m:m + 1])
                    g_, j_ = divmod(m, 4)
                    half_, j2_ = divmod(j_, 2)
                    nc.sync.dma_start(
                        out=xp_dram[half_, :, 2 * g_ + j2_, 512 * nb:512 * (nb + 1)],
                        in_=xpt)

            # ---- phase 2: recurrence ----
            c_prev = []
            for half in range(2):
                ct = smallp.tile([128, 64], F32, tag=f"c{half}")
                nc.vector.memset(ct, 0.0)
                c_prev.append(ct)
            if variant == "consth":
                nc.vector.memset(hh, 0.25)

            for t in range(T if do_p2 else 0):
                xph = []
                for half in range(2):
                    xt_ = xpchp.tile([128, 8, 32], BF16, tag=f"xpch{half}")
                    nc.sync.dma_start(out=xt_,
                                      in_=xp_dram[half, :, :, 32 * t:32 * (t + 1)])
                    xph.append(xt_)
                ps = []
                for half in range(2):
                    pst = psch.tile([128, 256], F32, tag=f"psch{half}")
                    if not v4:  # identity-matmul xp seed (v4 adds xp on DVE instead)
                        nc.tensor.matmul(pst, id_sb, xph[half],
                                         start=True, stop=(t == 0))
                    ps.append(pst)
                if t > 0 and variant != "nomm":
                    hprev = hh[:, 128 * (t - 1):128 * t]
                    for kp in range(2):  # k in {0,1} then {2,3}
                        for half in range(2):
                            for g in range(4):
                                for j2 in range(2):
                                    m = g * 4 + 2 * half + j2
                                    col = 64 * g + 32 * j2
                                    for k in (2 * kp, 2 * kp + 1):
                                        nc.tensor.matmul(
                                            ps[half][:, col:col + 32],
                                            wh_sb[:, k, 128 * m:128 * (m + 1)],
                                            hprev[:, 32 * k:32 * (k + 1)],
                                            start=(v4 and k == 0),
                                            stop=(k == KCH - 1))
                for half in range(2):
                    if v4:
                        if t == 0 or variant == "nomm":
                            gin = xph[half].rearrange("p g b -> p (g b)")
                        else:
                            ga = gsbp.tile([128, 256], F32, tag=f"ga{half}")
                            nc.vector.tensor_add(
                                ga, ps[half], xph[half].rearrange("p g b -> p (g b)"))
                            gin = ga
                    else:
                        gin = ps[half]
                    g_sb = gsbp.tile([128, 256], F32, tag=f"g{half}")
                    nc.scalar.activation(g_sb[:, 0:192], gin[:, 0:192], AF.Sigmoid)
                    nc.scalar.activation(g_sb[:, 192:256], gin[:, 192:256], AF.Tanh)
                    if variant == "consth":
                        if t == T - 1:
                            nc.sync.dma_start(
                                out=out_d[0:128, 256 * half:256 * (half + 1)],
                                in_=g_sb)
                        continue
                    t1 = smallp.tile([128, 64], F32, tag=f"t1{half}")
                    nc.vector.tensor_mul(t1, g_sb[:, 0:64], g_sb[:, 192:256])  # i*g
                    t2 = smallp.tile([128, 64], F32, tag=f"t2{half}")
                    if v2:  # f*c on the otherwise-idle GPSIMD engine
                        nc.gpsimd.tensor_mul(t2, g_sb[:, 64:128], c_prev[half])
                    else:
                        nc.vector.tensor_mul(t2, g_sb[:, 64:128], c_prev[half])
                    cn = smallp.tile([128, 64], F32, tag=f"c{half}")
                    nc.vector.tensor_add(cn, t1, t2)
                    th = smallp.tile([128, 64], F32, tag=f"th{half}")
                    nc.scalar.activation(th, cn, AF.Tanh)
                    nc.vector.tensor_mul(hh[:, 128 * t + 64 * half:128 * t + 64 * half + 64],
                                         g_sb[:, 128:192], th)                 # o*tanh(c)
                    c_prev[half] = cn

            # ---- phase 3: output projection ----
            hh_v = hh.rearrange("p (t j b) -> p t j b", j=KCH, b=32)
            for m in range(NOUT // 128 if do_p3 else 0):
                for nb in range(NB):
                    pso = psmm.tile([128, 512], F32, tag="psmm")
                    for j in range(KCH):
                        rhs = hh_v[:, 16 * nb:16 * (nb + 1), j, :]
                        nc.tensor.matmul(pso, we_sb[:, j, 128 * m:128 * (m + 1)],
                                         rhs, start=(j == 0), stop=(j == KCH - 1))
                    ot = osbp.tile([128, 512], F32, tag="osb")
                    if (m * NB + nb) % 2 == 0:
                        nc.scalar.copy(ot, pso)
                    else:
                        nc.vector.tensor_copy(ot, pso)
                    nc.sync.dma_start(
                        out=out_d[128 * m:128 * (m + 1), 512 * nb:512 * (nb + 1)], in_=ot)


def _build_program_v5(floor=False, loop_repeat=None, p1copy="dve", p3copy="act",
                      t2eng="pool", sigmerge=False, rec8=False,
                      wh8=False, h8=False, we8=False, mode="full", dr8=False):
    if rec8:
        wh8 = h8 = we8 = True
    if dr8:
        wh8 = True
    """Fused pipeline: phase-1 (input proj, for block b+2) and phase-3
    (output proj, for block b-1) matmuls are interleaved into the per-step
    recurrence issue stream so they execute during the elementwise-chain
    waits. xp lives in an SBUF ring (no DRAM spill, no per-step gather DMA).
    Same external I/O as the baseline program."""
    nc = bacc.Bacc("TRN2", target_bir_lowering=False, debug=False)
    WH_DT = FP8 if wh8 else BF16
    WE_DT = FP8 if we8 else BF16
    HH_DT = FP8 if h8 else BF16
    xT_d = nc.dram_tensor("xT", [NIN, TOK], BF16, kind="ExternalInput").ap()
    wi_d = nc.dram_tensor("wiT", [NIN, G], BF16, kind="ExternalInput").ap()
    wh_d = nc.dram_tensor("whT", [NH, G], WH_DT, kind="ExternalInput").ap()
    we_d = nc.dram_tensor("weT", [NH, NOUT], WE_DT, kind="ExternalInput").ap()
    b_d = nc.dram_tensor("biasM", [128, MT], F32, kind="ExternalInput").ap()
    i_d = nc.dram_tensor("ident", [128, 128], BF16, kind="ExternalInput").ap()
    out_d = nc.dram_tensor("outT", [NOUT, TOK], F32, kind="ExternalOutput").ap()

    NBLK = NB  # 8 token blocks; each = 16 steps x 32 batch = 512 tokens

    with tile.TileContext(nc) as tc:
        with tc.tile_pool(name="consts", bufs=1) as consts, \
             tc.tile_pool(name="xtk", bufs=3) as xtkp, \
             tc.tile_pool(name="ring", bufs=3) as ringp, \
             tc.tile_pool(name="gsb", bufs=3) as gsbp, \
             tc.tile_pool(name="small", bufs=3) as smallp, \
             tc.tile_pool(name="ost", bufs=2) as ostp, \
             tc.tile_pool(name="ps1", bufs=2, space="PSUM") as ps1p, \
             tc.tile_pool(name="ps3", bufs=2, space="PSUM") as ps3p, \
             tc.tile_pool(name="psr", bufs=2, space="PSUM") as psrp:

            wi_sb = consts.tile([128, KCH, G], BF16, tag="wi")
            nc.sync.dma_start(out=wi_sb, in_=wi_d.rearrange("(k p) m -> p k m", p=128))
            wh_sb = consts.tile([128, KCH, G], WH_DT, tag="wh")
            nc.sync.dma_start(out=wh_sb, in_=wh_d.rearrange("(k p) m -> p k m", p=128))
            we_sb = consts.tile([128, KCH, NOUT], WE_DT, tag="we")
            nc.sync.dma_start(out=we_sb, in_=we_d.rearrange("(j p) o -> p j o", p=128))
            b_sb = consts.tile([128, MT], F32, tag="bias")
            nc.sync.dma_start(out=b_sb, in_=b_d)
            id_sb = consts.tile([128, 128], BF16, tag="ident")
            nc.sync.dma_start(out=id_sb, in_=i_d)
            hh = consts.tile([128, T * 128], HH_DT, tag="hh")
            # fp8 copy of the h history: moving operand for DoubleRow
            # recurrence matmuls; bf16 hh still feeds phase 3.
            hh8 = consts.tile([128, T * 128], FP8, tag="hh8", name="hh8") if dr8 else None
            if mode == "consth":
                nc.vector.memset(hh, 0.25)
                if dr8:
                    nc.vector.memset(hh8, 0.25)

            if floor:
                t_ = gsbp.tile([128, 512], BF16, tag="t")
                nc.sync.dma_start(out=t_, in_=xT_d[0:128, 0:512])
                t2_ = ostp.tile([128, 512], F32, tag="t2")
                nc.vector.tensor_copy(t2_, t_)
                nc.sync.dma_start(out=out_d[0:128, 0:512], in_=t2_)
            else:
                import contextlib
                rep_cm = tc.For_i(0, loop_repeat, 1) if loop_repeat \
                    else contextlib.nullcontext()
                with rep_cm:
                    _emit_v5(nc, locals())
    nc.compile()
    return nc


def _emit_v5(nc, env):
    xT_d = env["xT_d"]; out_d = env["out_d"]
    wi_sb = env["wi_sb"]; wh_sb = env["wh_sb"]; we_sb = env["we_sb"]
    b_sb = env["b_sb"]; id_sb = env["id_sb"]; hh = env["hh"]
    xtkp = env["xtkp"]; ringp = env["ringp"]; gsbp = env["gsbp"]
    smallp = env["smallp"]; ostp = env["ostp"]
    ps1p = env["ps1p"]; ps3p = env["ps3p"]; psrp = env["psrp"]
    NBLK = env["NBLK"]
    p1copy = env["p1copy"]; p3copy = env["p3copy"]
    t2eng = env["t2eng"]; sigmerge = env["sigmerge"]; mode = env["mode"]
    dr8 = env["dr8"]; hh8 = env["hh8"]

    xT_r = xT_d.rearrange("(k p) t -> p k t", p=128)
    hh_v = hh.rearrange("p (t j b) -> p t j b", j=KCH, b=32)

    xt_t, ring_t = {}, {}

    def load_xt(b):
        if b >= NBLK:
            return
        xt_t[b] = xtkp.tile([128, KCH, 512], BF16, tag="xt", name=f"xt{b}")
        nc.sync.dma_start(out=xt_t[b], in_=xT_r[:, :, 512 * b:512 * (b + 1)])

    def p1_matmuls(b, m):
        """Input-projection m-tile m for block b: 4 PE matmuls into PSUM."""
        if b >= NBLK:
            return None
        if m == 0:
            ring_t[b] = ringp.tile([128, 2, 8, 512], BF16, tag="ring", name=f"ring{b}")
        psx = ps1p.tile([128, 512], F32, tag="ps1")
        for k in range(KCH):
            nc.tensor.matmul(psx, wi_sb[:, k, 128 * m:128 * (m + 1)],
                             xt_t[b][:, k, :], start=(k == 0), stop=(k == KCH - 1))
        return psx

    def p1_copy(b, m, psx):
        """PSUM -> xp ring (bf16) with bias add."""
        if psx is None:
            return
        g_, j_ = divmod(m, 4)
        half_, j2_ = divmod(j_, 2)
        dst = ring_t[b][:, half_, 2 * g_ + j2_, :]
        if p1copy == "act":
            nc.scalar.activation(dst, psx, AF.Identity, bias=b_sb[:, m:m + 1])
        elif p1copy == "pool":
            nc.gpsimd.tensor_scalar_add(dst, psx, b_sb[:, m:m + 1])
        else:
            nc.vector.tensor_scalar_add(dst, psx, b_sb[:, m:m + 1])

    ps3_cur = [None]

    def p3_matmul(b, j):
        """Output-projection work for block b, sub-step j (0..15)."""
        if b < 0 or b >= NBLK:
            return
        m, jj = divmod(j, 4)
        if jj == 0:
            ps3_cur[0] = ps3p.tile([128, 512], F32, tag="ps3", name=f"ps3_{b}_{m}")
        nc.tensor.matmul(ps3_cur[0], we_sb[:, jj, 128 * m:128 * (m + 1)],
                         hh_v[:, 16 * b:16 * (b + 1), jj, :],
                         start=(jj == 0), stop=(jj == KCH - 1))

    def p3_copy(b, j):
        if b < 0 or b >= NBLK:
            return
        m, jj = divmod(j, 4)
        if jj != KCH - 1:
            return
        ot = ostp.tile([128, 512], F32, tag="ost")
        if p3copy == "act" or (p3copy == "alt" and m % 2 == 0):
            nc.scalar.copy(ot, ps3_cur[0])
        else:
            nc.vector.tensor_copy(ot, ps3_cur[0])
        nc.sync.dma_start(
            out=out_d[128 * m:128 * (m + 1), 512 * b:512 * (b + 1)], in_=ot)

    # ---- prologue: x for blocks 0-2, input proj for blocks 0-1 ----
    load_xt(0); load_xt(1); load_xt(2)
    for b in (0, 1):
        for m in range(MT):
            p1_copy(b, m, p1_matmuls(b, m))

    c_prev = []
    for half in range(2):
        ct = smallp.tile([128, 64], F32, tag=f"c{half}")
        nc.vector.memset(ct, 0.0)
        c_prev.append(ct)

    # ---- fused main loop ----
    for t in range(T):
        blk, j = divmod(t, 16)
        if j == 0:
            load_xt(blk + 3)
        rt = ring_t[blk]
        c0 = 32 * j

        # PE stream: p1 (block blk+2), p3 (block blk-1) first -- no
        # dependence on the recurrent state, so they fill the chain wait.
        psx = p1_matmuls(blk + 2, j)
        p3_matmul(blk - 1, j)

        ps = []
        for half in range(2):
            pst = psrp.tile([128, 256], F32, tag=f"psr{half}")
            nc.tensor.matmul(pst, id_sb, rt[:, half, :, c0:c0 + 32],
                             start=True, stop=(t == 0 or mode == "nomm"))
            ps.append(pst)
        if t > 0 and mode != "nomm":
            if dr8:
                # DoubleRow fp8: one matmul per k-pair (2 packed k-planes)
                hp8 = hh8.rearrange("p (tt k b) -> p tt k b", k=KCH, b=32)
                for kp in range(2):
                    for half in range(2):
                        for g in range(4):
                            for j2 in range(2):
                                m = g * 4 + 2 * half + j2
                                col = 64 * g + 32 * j2
                                nc.tensor.matmul(
                                    ps[half][:, col:col + 32],
                                    wh_sb[:, 2 * kp:2 * kp + 2,
                                          128 * m:128 * (m + 1)],
                                    hp8[:, t - 1, 2 * kp:2 * kp + 2, :],
                                    start=False, stop=(kp == 1),
                                    perf_mode=mybir.MatmulPerfMode.DoubleRow)
            else:
                hprev = hh[:, 128 * (t - 1):128 * t]
                for kp in range(2):  # kp0 needs only h half0; kp1 only half1
                    for half in range(2):
                        for g in range(4):
                            for j2 in range(2):
                                m = g * 4 + 2 * half + j2
                                col = 64 * g + 32 * j2
                                for k in (2 * kp, 2 * kp + 1):
                                    nc.tensor.matmul(
                                        ps[half][:, col:col + 32],
                                        wh_sb[:, k, 128 * m:128 * (m + 1)],
                                        hprev[:, 32 * k:32 * (k + 1)],
                                        start=False, stop=(k == KCH - 1))

        # elementwise chain (ACT in half-order; DVE t1s before cns before hs)
        g_sb, t1s, cns = [], [], []
        for half in range(2):
            gs = gsbp.tile([128, 256], F32, tag=f"g{half}")
            if sigmerge:
                nc.scalar.activation(gs, ps[half], AF.Sigmoid)
            else:
                nc.scalar.activation(gs[:, 0:192], ps[half][:, 0:192], AF.Sigmoid)
                nc.scalar.activation(gs[:, 192:256], ps[half][:, 192:256], AF.Tanh)
            g_sb.append(gs)
        if mode == "consth":
            p1_copy(blk + 2, j, psx)
            p3_copy(blk - 1, j)
            continue
        for half in range(2):
            t1 = smallp.tile([128, 64], F32, tag=f"t1{half}")
            # sigmerge: t1' = sig(i) * sig(2g);  tanh recovered in cn below
            nc.vector.tensor_mul(t1, g_sb[half][:, 0:64], g_sb[half][:, 192:256])
            t1s.append(t1)
        t2s = []
        for half in range(2):
            t2 = smallp.tile([128, 64], F32, tag=f"t2{half}")
            if t2eng == "pool":
                nc.gpsimd.tensor_mul(t2, g_sb[half][:, 64:128], c_prev[half])
            else:
                nc.vector.tensor_mul(t2, g_sb[half][:, 64:128], c_prev[half])
            t2s.append(t2)
        qs = []
        if sigmerge:
            for half in range(2):
                # q = f*c - sig(i), so c' = 2*t1' + q  (Pool, off DVE path)
                q = smallp.tile([128, 64], F32, tag=f"q{half}")
                nc.gpsimd.tensor_sub(q, t2s[half], g_sb[half][:, 0:64])
                qs.append(q)
        for half in range(2):
            cn = smallp.tile([128, 64], F32, tag=f"c{half}")
            if sigmerge:
                nc.vector.scalar_tensor_tensor(
                    cn, t1s[half], 2.0, qs[half],
                    mybir.AluOpType.mult, mybir.AluOpType.add)
            else:
                nc.vector.tensor_add(cn, t1s[half], t2s[half])
            cns.append(cn)
        ths = []
        for half in range(2):
            th = smallp.tile([128, 64], F32, tag=f"th{half}")
            nc.scalar.activation(th, cns[half], AF.Tanh)
            ths.append(th)
        for half in range(2):
            if dr8:
                # critical-path write: fp8 h for next step's matmuls
                nc.vector.tensor_mul(
                    hh8[:, 128 * t + 64 * half:128 * t + 64 * half + 64],
                    g_sb[half][:, 128:192], ths[half])
                # off-path duplicate in bf16 for phase 3 (idle Pool engine)
                nc.gpsimd.tensor_mul(
                    hh[:, 128 * t + 64 * half:128 * t + 64 * half + 64],
                    g_sb[half][:, 128:192], ths[half])
            else:
                nc.vector.tensor_mul(
                    hh[:, 128 * t + 64 * half:128 * t + 64 * half + 64],
                    g_sb[half][:, 128:192], ths[half])
            c_prev[half] = cns[half]

        # copies issued last so they sit behind the chain in engine queues
        p1_copy(blk + 2, j, psx)
        p3_copy(blk - 1, j)

    # ---- epilogue: output projection for the final block ----
    for j in range(16):
        p3_matmul(NBLK - 1, j)
        p3_copy(NBLK - 1, j)




# ---------------------------------------------------------------------------
# v6: time-sharded. 8 cores = 2 directions x 4 time-segments of 32 steps,
# FULL batch B=128 per core. Cores s>0 warm-start from zero state Wwarm
# steps early (LSTM state contraction ~0.55/step makes the error ~3e-4 by
# the segment start); segment 0 instead runs Wwarm masked steps (i/o gates
# forced to ~0 by a -40 preactivation term) so its state at the segment
# start is exactly zero. Each step computes xp = W_ih@x_t directly into the
# gates PSUM (no xp ring / copies); h is consumed by the output projection
# one step later, so only a 3-deep h ring is kept.
NSEG = 4
SEG = T // NSEG  # 32


def _build_program_v6(floor=False, loop_repeat=None, sigmerge=False, Wwarm=16,
                      t2eng="pool", p3copy="dve"):
    NSTEP = SEG + Wwarm
    nc = bacc.Bacc("TRN2", target_bir_lowering=False, debug=False)
    xT_d = nc.dram_tensor("xT", [NIN, NSTEP * B], BF16, kind="ExternalInput").ap()
    wi_d = nc.dram_tensor("wiT", [NIN, G], BF16, kind="ExternalInput").ap()
    wh_d = nc.dram_tensor("whT", [NH, G], BF16, kind="ExternalInput").ap()
    we_d = nc.dram_tensor("weT", [NH, NOUT], BF16, kind="ExternalInput").ap()
    br_d = nc.dram_tensor("biasRow", [1, G], BF16, kind="ExternalInput").ap()
    wr_d = nc.dram_tensor("warmRow", [1, NSTEP * B], BF16, kind="ExternalInput").ap()
    on_d = nc.dram_tensor("onesRow", [1, B], BF16, kind="ExternalInput").ap()
    out_d = nc.dram_tensor("outT", [NOUT, NSTEP * B], F32, kind="ExternalOutput").ap()

    with tile.TileContext(nc) as tc:
        with tc.tile_pool(name="consts", bufs=1) as consts, \
             tc.tile_pool(name="xtk", bufs=3) as xtkp, \
             tc.tile_pool(name="gsb", bufs=3) as gsbp, \
             tc.tile_pool(name="hring", bufs=3) as hrp, \
             tc.tile_pool(name="small", bufs=3) as smallp, \
             tc.tile_pool(name="ost", bufs=2) as ostp, \
             tc.tile_pool(name="psg", bufs=3, space="PSUM") as psgp, \
             tc.tile_pool(name="ps3", bufs=2, space="PSUM") as ps3p:

            wi_sb = consts.tile([128, KCH, G], BF16, tag="wi")
            nc.sync.dma_start(out=wi_sb, in_=wi_d.rearrange("(k p) m -> p k m", p=128))
            wh_sb = consts.tile([128, KCH, G], BF16, tag="wh")
            nc.sync.dma_start(out=wh_sb, in_=wh_d.rearrange("(k p) m -> p k m", p=128))
            we_sb = consts.tile([128, KCH, NOUT], BF16, tag="we")
            nc.sync.dma_start(out=we_sb, in_=we_d.rearrange("(j p) o -> p j o", p=128))
            br_sb = consts.tile([1, G], BF16, tag="biasRow")
            nc.sync.dma_start(out=br_sb, in_=br_d)
            wr_sb = consts.tile([1, NSTEP * B], BF16, tag="warmRow")
            nc.sync.dma_start(out=wr_sb, in_=wr_d)
            on_sb = consts.tile([1, B], BF16, tag="onesRow")
            nc.sync.dma_start(out=on_sb, in_=on_d)
            m40_sb = consts.tile([1, 128], BF16, tag="m40")
            nc.vector.memset(m40_sb, -40.0)

            if floor:
                t_ = gsbp.tile([128, 512], BF16, tag="t")
                nc.sync.dma_start(out=t_, in_=xT_d[0:128, 0:512])
                t2_ = ostp.tile([128, 512], F32, tag="t2")
                nc.vector.tensor_copy(t2_, t_)
                nc.sync.dma_start(out=out_d[0:128, 0:512], in_=t2_)
            else:
                import contextlib
                rep_cm = tc.For_i(0, loop_repeat, 1) if loop_repeat \
                    else contextlib.nullcontext()
                with rep_cm:
                    _emit_v6(nc, locals())
    nc.compile()
    return nc


def _emit_v6(nc, env):
    xT_d = env["xT_d"]; out_d = env["out_d"]
    wi_sb = env["wi_sb"]; wh_sb = env["wh_sb"]; we_sb = env["we_sb"]
    br_sb = env["br_sb"]; wr_sb = env["wr_sb"]; on_sb = env["on_sb"]
    m40_sb = env["m40_sb"]
    xtkp = env["xtkp"]; gsbp = env["gsbp"]; hrp = env["hrp"]
    smallp = env["smallp"]; ostp = env["ostp"]
    psgp = env["psgp"]; ps3p = env["ps3p"]
    sigmerge = env["sigmerge"]; t2eng = env["t2eng"]; p3copy = env["p3copy"]
    NSTEP = env["NSTEP"]

    xT_r = xT_d.rearrange("(k p) t -> p k t", p=128)
    NBLK6 = NSTEP // 4
    xt_t, hh_t, ps_t = {}, {}, {}

    def load_xt(b):
        if 0 <= b < NBLK6:
            xt_t[b] = xtkp.tile([128, KCH, 512], BF16, tag="xt", name=f"xt{b}")
            nc.sync.dma_start(out=xt_t[b], in_=xT_r[:, :, 512 * b:512 * (b + 1)])

    def xp_fill(tau, half):
        """xp + bias (+warm mask) matmuls for step tau, gate half."""
        if tau >= NSTEP:
            return
        if half == 0:
            ps_t[tau] = []
        pst = psgp.tile([128, 8, B], F32, tag=f"psg{half}",
                        name=f"psg{half}_{tau}")
        ps_t[tau].append(pst)
        xt = xt_t[tau // 4][:, :, 128 * (tau % 4):128 * (tau % 4) + 128]
        for g in range(4):
            for j2 in range(2):
                m = g * 4 + 2 * half + j2
                reg = pst[:, 2 * g + j2, :]
                for k in range(KCH):
                    nc.tensor.matmul(reg, wi_sb[:, k, 128 * m:128 * (m + 1)],
                                     xt[:, k, :], start=(k == 0), stop=False)
                nc.tensor.matmul(reg, br_sb[:, 128 * m:128 * (m + 1)],
                                 on_sb, start=False, stop=False)
                is_io = g in (0, 2)  # i- and o-gate regions get the warm mask
                if is_io:
                    nc.tensor.matmul(reg, m40_sb,
                                     wr_sb[:, B * tau:B * (tau + 1)],
                                     start=False, stop=(tau == 0))
                elif tau == 0:
                    # close the group: harmless zero-valued mask add
                    nc.tensor.matmul(reg, m40_sb,
                                     wr_sb[:, 0:B].bitcast(BF16) if False else
                                     on_sb, start=False, stop=False)
        return pst

    def rec_matmuls(tau):
        if tau == 0:
            return
        hprev = hh_t[tau - 1]
        for kp in range(2):
            for half in range(2):
                for g in range(4):
                    for j2 in range(2):
                        m = g * 4 + 2 * half + j2
                        reg = ps_t[tau][half][:, 2 * g + j2, :]
                        for k in (2 * kp, 2 * kp + 1):
                            nc.tensor.matmul(
                                reg, wh_sb[:, k, 128 * m:128 * (m + 1)],
                                hprev[:, k, :], start=False,
                                stop=(k == KCH - 1))

    ps3_cur = [None]

    def p3_matmuls(tau):
        """Output projection for h(tau)."""
        if not (0 <= tau < NSTEP):
            return
        po = ps3p.tile([128, KCH, B], F32, tag="ps3", name=f"ps3_{tau}")
        ps3_cur[0] = po
        h = hh_t[tau]
        for mo in range(KCH):
            for k in range(KCH):
                nc.tensor.matmul(po[:, mo, :],
                                 we_sb[:, k, 128 * mo:128 * (mo + 1)],
                                 h[:, k, :], start=(k == 0), stop=(k == KCH - 1))

    def p3_copy_dma(tau):
        if not (0 <= tau < NSTEP):
            return
        ot = ostp.tile([128, KCH, B], F32, tag="ost", name=f"ost{tau}")
        if p3copy == "act":
            nc.scalar.copy(ot, ps3_cur[0])
        else:
            nc.vector.tensor_copy(ot, ps3_cur[0])
        nc.sync.dma_start(
            out=out_d.rearrange("(mo p) t -> p mo t", p=128)[:, :, B * tau:B * (tau + 1)],
            in_=ot)

    # ---- prologue ----
    load_xt(0); load_xt(1); load_xt(2)
    xp_fill(0, 0); xp_fill(0, 1)
    c_prev = []
    for half in range(2):
        ct = smallp.tile([128, 256], F32, tag=f"c{half}", name=f"c{half}")
        nc.vector.memset(ct, 0.0)
        c_prev.append(ct)

    # ---- main loop ----
    for tau in range(NSTEP):
        if tau % 4 == 0:
            load_xt(tau // 4 + 3)
        # PE: future-step xp (independent of h) first, then output
        # projection of h(tau-1), then the h-dependent recurrence matmuls.
        xp_fill(tau + 1, 0)
        p3_matmuls(tau - 1)
        rec_matmuls(tau)
        xp_fill(tau + 1, 1)

        ps = ps_t.pop(tau)
        hh_t[tau] = hrp.tile([128, KCH, B], BF16, tag="hh", name=f"hh{tau}")
        g_sb, t1s, t2s, cns, qs = [], [], [], [], []
        for half in range(2):
            gs = gsbp.tile([128, 8 * B], F32, tag=f"g{half}", name=f"g{half}_{tau}")
            gsv = gs.rearrange("p (m b) -> p m b", b=B)
            if sigmerge:
                nc.scalar.activation(gsv, ps[half], AF.Sigmoid)
            else:
                nc.scalar.activation(gsv[:, 0:6, :], ps[half][:, 0:6, :], AF.Sigmoid)
                nc.scalar.activation(gsv[:, 6:8, :], ps[half][:, 6:8, :], AF.Tanh)
            g_sb.append(gs)
        for half in range(2):
            t1 = smallp.tile([128, 256], F32, tag=f"t1{half}", name=f"t1{half}_{tau}")
            nc.vector.tensor_mul(t1, g_sb[half][:, 0:256], g_sb[half][:, 768:1024])
            t1s.append(t1)
        for half in range(2):
            t2 = smallp.tile([128, 256], F32, tag=f"t2{half}", name=f"t2{half}_{tau}")
            if t2eng == "pool":
                nc.gpsimd.tensor_mul(t2, g_sb[half][:, 256:512], c_prev[half])
            else:
                nc.vector.tensor_mul(t2, g_sb[half][:, 256:512], c_prev[half])
            t2s.append(t2)
        if sigmerge:
            for half in range(2):
                q = smallp.tile([128, 256], F32, tag=f"q{half}", name=f"q{half}_{tau}")
                nc.gpsimd.tensor_sub(q, t2s[half], g_sb[half][:, 0:256])
                qs.append(q)
        for half in range(2):
            cn = smallp.tile([128, 256], F32, tag=f"c{half}", name=f"cn{half}_{tau}")
            if sigmerge:
                nc.vector.scalar_tensor_tensor(
                    cn, t1s[half], 2.0, qs[half],
                    mybir.AluOpType.mult, mybir.AluOpType.add)
            else:
                nc.vector.tensor_add(cn, t1s[half], t2s[half])
            cns.append(cn)
        ths = []
        for half in range(2):
            th = smallp.tile([128, 256], F32, tag=f"th{half}", name=f"th{half}_{tau}")
            nc.scalar.activation(th, cns[half], AF.Tanh)
            ths.append(th)
        for half in range(2):
            nc.vector.tensor_mul(
                hh_t[tau][:, 2 * half:2 * half + 2, :].rearrange("p k b -> p (k b)"),
                g_sb[half][:, 512:768], ths[half])
            c_prev[half] = cns[half]
        p3_copy_dma(tau - 1)
        hh_t.pop(tau - 2, None)

    p3_matmuls(NSTEP - 1)
    p3_copy_dma(NSTEP - 1)


def make_in_maps_v6(x, W_ih_f, W_hh_f, b_ih_f, b_hh_f,
                    W_ih_r, W_hh_r, b_ih_r, b_hh_r, W_emb, b_emb,
                    sigmerge=None, Wwarm=16):
    f32 = np.float32
    cfg = _kvar_cfg()
    if sigmerge is None:
        sigmerge = cfg.get("sigmerge", False)
    NSTEP = SEG + Wwarm

    def dir_weights(W_ih, W_hh, b_ih, b_hh, we_cols):
        wi = W_ih.astype(f32)[_PERM]
        wh = W_hh.astype(f32)[_PERM]
        bias = (b_ih.astype(f32) + b_hh.astype(f32))[_PERM]
        if sigmerge:
            wi = wi.copy(); wh = wh.copy(); bias = bias.copy()
            wi[1536:2048] *= 2.0
            wh[1536:2048] *= 2.0
            bias[1536:2048] *= 2.0
        wiT = np.ascontiguousarray(wi.T).astype(BF16_NP)
        whT = np.ascontiguousarray(wh.T).astype(BF16_NP)
        biasRow = bias.reshape(1, G).astype(BF16_NP)
        weT = np.ascontiguousarray(we_cols.astype(f32).T).astype(BF16_NP)
        return wiT, whT, biasRow, weT

    wf = dir_weights(W_ih_f, W_hh_f, b_ih_f, b_hh_f, W_emb[:, :NH])
    wr = dir_weights(W_ih_r, W_hh_r, b_ih_r, b_hh_r, W_emb[:, NH:])
    onesRow = np.ones((1, B), BF16_NP)

    x_f32 = x.astype(f32)
    in_maps = []
    for core in range(NCORES):
        direction, s = divmod(core, NSEG)
        xs = x_f32 if direction == 0 else x_f32[::-1]
        t0 = SEG * s - Wwarm
        xseg = np.zeros((NSTEP, B, NIN), f32)
        lo = max(t0, 0)
        xseg[lo - t0:] = xs[lo:SEG * s + SEG]
        xT = np.ascontiguousarray(
            xseg.transpose(2, 0, 1).reshape(NIN, NSTEP * B)).astype(BF16_NP)
        warmRow = np.zeros((1, NSTEP * B), BF16_NP)
        if s == 0:
            warmRow[0, :Wwarm * B] = 1.0
        wiT, whT, biasRow, weT = wf if direction == 0 else wr
        in_maps.append({"xT": xT, "wiT": wiT, "whT": whT, "weT": weT,
                        "biasRow": biasRow, "warmRow": warmRow,
                        "onesRow": onesRow})
    return in_maps


def assemble_output_v6(results, b_emb, Wwarm=16):
    NSTEP = SEG + Wwarm
    out = np.empty((T, B, NOUT), np.float32)
    for s in range(NSEG):
        pf = results[s]["outT"].reshape(NOUT, NSTEP, B)[:, Wwarm:, :]
        out[SEG * s:SEG * (s + 1)] = pf.transpose(1, 2, 0)
    for s in range(NSEG):
        pr = results[NSEG + s]["outT"].reshape(NOUT, NSTEP, B)[:, Wwarm:, :]
        # reverse core s covers flipped steps [SEG*s, SEG*s+SEG) ->
        # original steps [T-1-SEG*s-(SEG-1), T-SEG*s)
        blk = pr[:, ::-1, :].transpose(1, 2, 0)  # original time order
        out[T - SEG * (s + 1):T - SEG * s] += blk
    out += b_emb.astype(np.float32)
    return out


_NC_CACHE = None


def _kvar_cfg():
    """Parse KVAR env: "v5" (default), "v0" (old baseline), "v5:wh8=1,..."."""
    kv = os.environ.get("KVAR", "v5")
    if kv == "v0":
        kv = ""
    if kv == "v5f8":
        kv = "v5:wh8=1,h8=1,we8=1"
    if not kv.startswith("v5"):
        return {"v5": False}
    cfg = {"v5": True}
    if ":" in kv:
        for kvp in kv.split(":", 1)[1].split(","):
            key, val = kvp.split("=")
            cfg[key] = (int(val) if val.isdigit() else val)
    return cfg


def _get_nc():
    global _NC_CACHE
    if _NC_CACHE is None:
        cfg = _kvar_cfg()
        if cfg.pop("v5"):
            _NC_CACHE = _build_program_v5(**cfg)
        else:
            _NC_CACHE = _build_program()
    return _NC_CACHE


def _build_for_timing(loop_repeat=None, floor=False):
    cfg = _kvar_cfg()
    if cfg.pop("v5"):
        return _build_program_v5(loop_repeat=loop_repeat, floor=floor, **cfg)
    return _build_program(loop_repeat=loop_repeat, floor=floor)


def make_in_maps(x, W_ih_f, W_hh_f, b_ih_f, b_hh_f,
                 W_ih_r, W_hh_r, b_ih_r, b_hh_r, W_emb, b_emb,
                 wh8=None, we8=None, sigmerge=None):
    """Host-side sharding/layout prep -> per-core input maps (8 cores)."""
    f32 = np.float32
    cfg = _kvar_cfg()
    if wh8 is None:
        wh8 = cfg.get("wh8", False)
    if we8 is None:
        we8 = cfg.get("we8", False)
    if sigmerge is None:
        sigmerge = cfg.get("sigmerge", False)
    WH_NP = F8_NP if wh8 else BF16_NP
    WE_NP = F8_NP if we8 else BF16_NP

    def dir_weights(W_ih, W_hh, b_ih, b_hh, we_cols):
        wi = W_ih.astype(f32)[_PERM]
        wh = W_hh.astype(f32)[_PERM]
        bias = (b_ih.astype(f32) + b_hh.astype(f32))[_PERM]
        if sigmerge:
            # g-gate preactivations scaled by 2 so sigmoid covers all four
            # gate regions in one pass: tanh(x) = 2*sigmoid(2x) - 1
            wi = wi.copy(); wh = wh.copy(); bias = bias.copy()
            wi[1536:2048] *= 2.0
            wh[1536:2048] *= 2.0
            bias[1536:2048] *= 2.0
        wiT = np.ascontiguousarray(wi.T).astype(BF16_NP)
        whT = np.ascontiguousarray(wh.T).astype(WH_NP)
        biasM = np.ascontiguousarray(bias.reshape(MT, 128).T)
        weT = np.ascontiguousarray(we_cols.astype(f32).T).astype(WE_NP)
        return wiT, whT, biasM, weT

    wf = dir_weights(W_ih_f, W_hh_f, b_ih_f, b_hh_f, W_emb[:, :NH])
    wr = dir_weights(W_ih_r, W_hh_r, b_ih_r, b_hh_r, W_emb[:, NH:])
    ident = np.eye(128, dtype=BF16_NP)

    x_f32 = x.astype(f32)
    in_maps = []
    for core in range(NCORES):
        direction, q = divmod(core, 4)
        xs = x_f32[:, BL * q:BL * (q + 1), :]
        if direction == 1:
            xs = xs[::-1]
        # xT[feat, t*BL + b] = xs[t, b, feat]
        xT = np.ascontiguousarray(xs.transpose(2, 0, 1).reshape(NIN, TOK)).astype(BF16_NP)
        wiT, whT, biasM, weT = wf if direction == 0 else wr
        in_maps.append({"xT": xT, "wiT": wiT, "whT": whT, "weT": weT,
                        "biasM": biasM, "ident": ident})
    return in_maps


def assemble_output(results, b_emb):
    """Combine 8 per-core outT partials into the full (T, B, NOUT) output."""
    out = np.empty((T, B, NOUT), np.float32)
    for q in range(4):
        pf = results[q]["outT"].reshape(NOUT, T, BL).transpose(1, 2, 0)
        pr = results[4 + q]["outT"].reshape(NOUT, T, BL)[:, ::-1, :].transpose(1, 2, 0)
        out[:, BL * q:BL * (q + 1), :] = pf + pr
    out += b_emb.astype(np.float32)
    return out


def kernel(x, W_ih_f, W_hh_f, b_ih_f, b_hh_f,
           W_ih_r, W_hh_r, b_ih_r, b_hh_r, W_emb, b_emb):
    nc = _get_nc()
    in_maps = make_in_maps(x, W_ih_f, W_hh_f, b_ih_f, b_hh_f,
                           W_ih_r, W_hh_r, b_ih_r, b_hh_r, W_emb, b_emb)
    res = run_bass_kernel_spmd(nc, in_maps, list(range(NCORES)))
    return assemble_output(res.results, b_emb)

